# revision 13
# baseline (speedup 1.0000x reference)
"""Multi-head attention + residual + LayerNorm on 8 TRN2 NeuronCores.

Problem (fixed shapes): B=2, S=2048, D=1024, H=16 heads, head_dim=64.
    q,k,v = x@Wq+bq, x@Wk+bk, x@Wv+bv   (per-head split)
    probs = softmax(q@k^T/8 + mask); ctx = probs@v
    out = LayerNorm(ctx@Wo + bo + x) * gamma + beta

Stage A (tensor-parallel over heads): core c owns heads {2c, 2c+1}.
  - q/k/v projections in fp8 (e4m3) with DoubleRow matmuls (256-wide
    contraction per instruction, 2x bf16 FLOP rate). Weights are host-scaled
    by 8 so the PSUM->SBUF copies are plain dtype-converting tensor_copy.
  - v is produced transposed (feature-major, FD=512 streams), then flipped
    to [key, feat] via PE-transpose; the [*,*,64] column of v_all is a ones
    column producing the softmax denominator for free during ctx matmuls
    (cols 65..79 are zero padding so the DoubleRow stationary is 80 wide,
    16-aligned -- 65-wide DR stationaries hang the PE).
  - scores: bf16, contraction 64 per head; the two heads' matmuls are
    issued to disjoint PE row groups (tile_position (0,0)/(64,0)) and run
    concurrently (~1.7x).
  - softmax exp: split across the Scalar engine (true Exp activation,
    fp8 out) and the Vector engine (1-instruction Schraudolph exp:
    int8(score*8*log2e/512 + C) bit-cast as e4m3). The softmax
    normalization divides out the ~3-8% approximation error; what remains
    is diluted ~100x by the residual path.
  - ctx: fp8 DoubleRow over key-tile pairs (2 key tiles per matmul).
  - Output: unnormalized ctx (8*sum p~*v) + denominator row, bf16.
Stage B (data-parallel over rows): core c owns rows [512c, 512(c+1)).
  - The softmax denominator division rides along as a per-(head,query)
    reciprocal tensor (host reshuffles den -> recb); ctr_n = ctr*recb on
    GPSIMD, then ctx@Wo in fp8 DoubleRow, +residual, LayerNorm (bn_stats
    on DVE, final affine on the Scalar engine via Identity activation).
Host only reshuffles/casts arrays between stages (plus 65k reciprocals
for the denominator and the x+bo add, both O(R*D) \ll attention FLOPs).
"""

import numpy as np
import ml_dtypes

import concourse.bacc as bacc
import concourse.bass as bass
import concourse.tile as tile
from concourse import mybir
from concourse.bass_utils import run_bass_kernel_spmd

BF16 = ml_dtypes.bfloat16
E4M3 = ml_dtypes.float8_e4m3

B, S, D, H = 2, 2048, 1024, 16
HD = D // H          # 64
NCORES = 8
HPC = H // NCORES    # 2 heads per core
R = B * S            # 4096 rows
RPC = R // NCORES    # 512 rows per core in stage B
LN_EPS = 1e-12

LOG2E = 1.4426950408889634
SCH_C = 0.395            # Schraudolph constant tuned for RNE-int8+e4m3
SCH_MUL = 8 * LOG2E / 512
SCH_ADD = 56.0 + 8 * SCH_C
ACT_NUM, ACT_DEN = 73, 128  # fraction of exp units on the Scalar engine

_cache = {}
PROFILE = False
last_exec_ns = {}


def _build_stage_a(qkv_trivial=True):
    nc = bacc.Bacc("TRN2", target_bir_lowering=False, debug=False,
                   num_devices=NCORES)
    f32 = mybir.dt.float32
    bf16 = mybir.dt.bfloat16
    fp8 = mybir.dt.float8e4
    DR = mybir.MatmulPerfMode.DoubleRow

    xts = nc.dram_tensor("xts", [8, 128, 4, 2, 512], fp8,
                         kind="ExternalInput").ap()
    wqk = nc.dram_tensor("wqk", [128, 4, 2, 256], fp8,
                         kind="ExternalInput").ap()
    wv = nc.dram_tensor("wv", [128, 4, 2, 128], fp8,
                        kind="ExternalInput").ap()
    bqk = nc.dram_tensor("bqk", [128, 2], f32, kind="ExternalInput").ap()
    bvp = nc.dram_tensor("bv", [128, 1], f32, kind="ExternalInput").ap()
    mask_e = nc.dram_tensor("mask_e", [128, B, 16], f32,
                            kind="ExternalInput").ap()
    mask_s = nc.dram_tensor("mask_s", [128, B, 16], f32,
                            kind="ExternalInput").ap()
    id8 = nc.dram_tensor("id8", [128, 128], bf16, kind="ExternalInput").ap()
    ctxT = nc.dram_tensor("ctxT", [HPC, 65, R], f32,
                          kind="ExternalOutput").ap()

    with tile.TileContext(nc) as tc:
        with (
            tc.tile_pool(name="singles", bufs=1) as singles,
            tc.tile_pool(name="xt", bufs=3) as xt_pool,
            tc.tile_pool(name="vt", bufs=2) as vt_pool,
            tc.tile_pool(name="pps", bufs=1, space="PSUM") as proj_ps,
            tc.tile_pool(name="ex", bufs=5) as ex_pool,
            tc.tile_pool(name="ob", bufs=3) as ob_pool,
        ):
            w_qk = singles.tile([128, 4, 2, 256], fp8)
            nc.sync.dma_start(out=w_qk, in_=wqk[:])
            w_v = singles.tile([128, 4, 2, 128], fp8)
            nc.sync.dma_start(out=w_v, in_=wv[:])
            bqk_sb = singles.tile([128, 2], f32)
            nc.sync.dma_start(out=bqk_sb, in_=bqk[:])
            bv_sb = singles.tile([128, 1], f32)
            nc.sync.dma_start(out=bv_sb, in_=bvp[:])
            me_sb = singles.tile([128, B, 16], f32)
            nc.sync.dma_start(out=me_sb, in_=mask_e[:])
            ms_sb = singles.tile([128, B, 16], f32)
            nc.sync.dma_start(out=ms_sb, in_=mask_s[:])
            id_sb = singles.tile([128, 128], bf16)
            nc.sync.dma_start(out=id_sb, in_=id8[:])

            q_sb = singles.tile([128, R], bf16)
            k_sb = singles.tile([128, R], bf16)
            # [key_part, b, key-tile-pair, pair-slab, h*80+feat]
            v_all = singles.tile([128, B, 8, 2, 160], fp8)
            nc.vector.memset(v_all, 0.0)
            nc.vector.memset(v_all[:, :, :, :, 64:65], 1.0)
            nc.vector.memset(v_all[:, :, :, :, 144:145], 1.0)

            exp_idx = [0]

            TAGS = [("sc0", 3), ("sc1", 2), ("ctx0", 1), ("ctx1", 1),
                    ("proj", 1)]

            def qkv_pieces(rb, spread=False):
                b = rb // 4
                tag_i = [0]

                def ptag():
                    if not spread:
                        return ("proj", 1)
                    t = TAGS[tag_i[0] % len(TAGS)]
                    tag_i[0] += 1
                    return t

                xt = xt_pool.tile([128, 4, 2, 512], fp8, tag="xt",
                                  name=f"xt_{rb}")
                nc.sync.dma_start(out=xt[:, 0:2], in_=xts[rb, :, 0:2])
                nc.gpsimd.dma_start(out=xt[:, 2:4], in_=xts[rb, :, 2:4])

                def proj(col_lo, dst_sb, bias_col):
                    tg, bf = ptag()

                    def _p():
                        ps = proj_ps.tile([128, 512], f32, tag=tg, bufs=bf,
                                          name=f"pj_{rb}_{col_lo}")
                        for k2 in range(4):
                            nc.tensor.matmul(
                                ps,
                                lhsT=w_qk[:, k2, :, col_lo:col_lo + 128],
                                rhs=xt[:, k2],
                                start=(k2 == 0), stop=(k2 == 3),
                                perf_mode=DR)
                        dst = dst_sb[:, rb * 512:(rb + 1) * 512]
                        if qkv_trivial:
                            nc.vector.tensor_copy(out=dst, in_=ps)
                        else:
                            nc.vector.tensor_scalar(
                                out=dst, in0=ps, scalar1=1.0,
                                scalar2=bqk_sb[:, bias_col:bias_col + 1],
                                op0=mybir.AluOpType.mult,
                                op1=mybir.AluOpType.add)
                    return _p

                vt = vt_pool.tile([128, 512], bf16, tag="vt",
                                  name=f"vt_{rb}")

                vtg, vbf = ptag()

                def vproj():
                    ps = proj_ps.tile([128, 512], f32, tag=vtg, bufs=vbf,
                                      name=f"pv_{rb}")
                    for k2 in range(4):
                        nc.tensor.matmul(
                            ps, lhsT=w_v[:, k2], rhs=xt[:, k2],
                            start=(k2 == 0), stop=(k2 == 3), perf_mode=DR)
                    if qkv_trivial:
                        nc.vector.tensor_copy(out=vt, in_=ps)
                    else:
                        nc.vector.tensor_scalar(
                            out=vt, in0=ps, scalar1=1.0, scalar2=bv_sb,
                            op0=mybir.AluOpType.mult,
                            op1=mybir.AluOpType.add)

                def vtrans(rt):
                    tg, bf = ptag()

                    def _t():
                        trp = proj_ps.tile([128, 128], bf16, tag=tg, bufs=bf,
                                           name=f"tr_{rb}_{rt}")
                        nc.tensor.transpose(
                            trp, vt[:, rt * 128:(rt + 1) * 128], id_sb)
                        jt = (rb % 4) * 4 + rt
                        dst = v_all[:, b, jt // 2, jt % 2].rearrange(
                            "p (h c) -> p h c", h=2)[:, :, 0:64]
                        src = trp.rearrange("p (h c) -> p h c", h=2)
                        if rt % 2 == 0:
                            nc.vector.tensor_copy(out=dst, in_=src)
                        else:
                            nc.scalar.copy(out=dst, in_=src)
                    return _t

                return [proj(0, q_sb, 0), proj(128, k_sb, 1), vproj,
                        vtrans(0), vtrans(1), vtrans(2), vtrans(3)]

            def emit_attention(b, ib, fillers=()):
                q_lo = b * S + ib * 512
                fillers = list(fillers)
                cps = [proj_ps.tile([80, 512], f32, tag=f"ctx{h}", bufs=1,
                                   name=f"ctx_{b}_{ib}_{h}")
                       for h in range(HPC)]
                def emit_ctx(jtp, exs):
                    for h in range(HPC):
                        nc.tensor.matmul(
                            cps[h],
                            lhsT=v_all[:, b, jtp, :,
                                       h * 80:(h + 1) * 80],
                            rhs=exs[:, :, h],
                            start=(jtp == 0), stop=(jtp == 7),
                            perf_mode=DR)

                pending = []
                for jtp in range(8):
                    if pending:
                        emit_ctx(jtp - 1, pending.pop(0))
                    if fillers:
                        fillers.pop(0)()
                    ex = ex_pool.tile([128, 2, 2, 512], fp8, tag="ex",
                                      name=f"ex_{b}_{ib}_{jtp}")
                    for sl in range(2):
                        jt = jtp * 2 + sl
                        k_lo = b * S + jt * 128
                        s0 = proj_ps.tile([128, 512], f32, tag="sc0", bufs=3,
                                          name=f"s0_{b}_{ib}_{jt}")
                        s1 = proj_ps.tile([128, 512], f32, tag="sc1", bufs=2,
                                          name=f"s1_{b}_{ib}_{jt}")
                        nc.tensor.matmul(
                            s0, lhsT=k_sb[0:64, k_lo:k_lo + 128],
                            rhs=q_sb[0:64, q_lo:q_lo + 512],
                            start=True, stop=True, tile_position=(0, 0))
                        nc.tensor.matmul(
                            s1, lhsT=k_sb[64:128, k_lo:k_lo + 128],
                            rhs=q_sb[64:128, q_lo:q_lo + 512],
                            start=True, stop=True, tile_position=(64, 0))
                        for h, sps in ((0, s0), (1, s1)):
                            if (jt + h) % 2 == 0:
                                nc.scalar.activation(
                                    out=ex[:, sl, h], in_=sps,
                                    func=mybir.ActivationFunctionType.Exp,
                                    bias=me_sb[:, b, jt:jt + 1],
                                    scale=1.0 / 512)
                            else:
                                nc.vector.tensor_scalar(
                                    out=ex[:, sl, h].bitcast(mybir.dt.int8),
                                    in0=sps, scalar1=SCH_MUL,
                                    scalar2=ms_sb[:, b, jt:jt + 1],
                                    op0=mybir.AluOpType.mult,
                                    op1=mybir.AluOpType.add)
                    pending.append(ex)
                emit_ctx(7, pending.pop(0))
                for h in range(HPC):
                    ob = ob_pool.tile([65, 512], f32, tag="ob")
                    if h == 0:
                        nc.scalar.copy(out=ob, in_=cps[h][0:65, :])
                    else:
                        nc.vector.tensor_copy(out=ob, in_=cps[h][0:65, :])
                    eng = nc.sync if h == 0 else nc.gpsimd
                    eng.dma_start(out=ctxT[h, :, q_lo:q_lo + 512], in_=ob)

            for rb in range(4):
                for piece in qkv_pieces(rb, spread=True):
                    piece()
            for ib in range(4):
                emit_attention(0, ib, fillers=qkv_pieces(4 + ib))
            for ib in range(4):
                emit_attention(1, ib)

    nc.compile()
    return nc


def _build_stage_b(ln_trivial=True):
    nc = bacc.Bacc("TRN2", target_bir_lowering=False, debug=False,
                   num_devices=NCORES)
    f32 = mybir.dt.float32
    bf16 = mybir.dt.bfloat16
    fp8 = mybir.dt.float8e4
    DR = mybir.MatmulPerfMode.DoubleRow

    ctr = nc.dram_tensor("ctr", [128, 4, 2, 512], fp8,
                         kind="ExternalInput").ap()
    recb = nc.dram_tensor("recb", [128, 4, 2, 512], fp8,
                          kind="ExternalInput").ap()
    wo = nc.dram_tensor("wo", [128, 4, 2, 1024], fp8,
                        kind="ExternalInput").ap()
    xpb = nc.dram_tensor("xpb", [RPC, D], bf16, kind="ExternalInput").ap()
    gamma = nc.dram_tensor("gamma", [D], f32, kind="ExternalInput").ap()
    beta = nc.dram_tensor("beta", [D], f32, kind="ExternalInput").ap()
    out = nc.dram_tensor("out", [RPC, D], bf16, kind="ExternalOutput").ap()

    with tile.TileContext(nc) as tc:
        with (
            tc.tile_pool(name="singles", bufs=1) as singles,
            tc.tile_pool(name="xp", bufs=2) as xp_pool,
            tc.tile_pool(name="hid", bufs=2) as h_pool,
            tc.tile_pool(name="ps", bufs=2, space="PSUM") as ps_pool,
            tc.tile_pool(name="stat", bufs=4) as stat_pool,
            tc.tile_pool(name="outp", bufs=2) as out_pool,
        ):
            engs = [nc.sync, nc.scalar, nc.gpsimd]
            wo_sb = singles.tile([128, 4, 2, 1024], fp8)
            ctr_sb = singles.tile([128, 4, 2, 512], fp8)
            rec_sb = singles.tile([128, 4, 2, 512], fp8)
            ctrn = singles.tile([128, 4, 2, 512], fp8)
            for k2 in range(4):
                engs[k2 % 2].dma_start(out=ctr_sb[:, k2], in_=ctr[:, k2])
                engs[(k2 + 1) % 2].dma_start(out=rec_sb[:, k2],
                                             in_=recb[:, k2])
            for k2 in range(4):
                engs[2].dma_start(out=wo_sb[:, k2], in_=wo[:, k2])
            gm_sb = singles.tile([128, D], f32)
            nc.sync.dma_start(
                out=gm_sb,
                in_=bass.AP(tensor=gamma.tensor, offset=gamma.offset,
                            ap=[[0, 128]] + list(gamma.ap)))
            bt_sb = singles.tile([128, D], f32)
            nc.sync.dma_start(
                out=bt_sb,
                in_=bass.AP(tensor=beta.tensor, offset=beta.offset,
                            ap=[[0, 128]] + list(beta.ap)))
            eps_sb = singles.tile([128, 1], f32)
            nc.vector.memset(eps_sb, LN_EPS)

            MT = RPC // 128  # 4 row tiles
            for mt in range(MT):
                nc.vector.tensor_mul(
                    out=ctrn[:, :, :, mt * 128:(mt + 1) * 128],
                    in0=ctr_sb[:, :, :, mt * 128:(mt + 1) * 128],
                    in1=rec_sb[:, :, :, mt * 128:(mt + 1) * 128])
            for mt in range(MT):
                xp = xp_pool.tile([128, D], bf16)
                nc.sync.dma_start(out=xp,
                                  in_=xpb[mt * 128:(mt + 1) * 128, :])
                hid = h_pool.tile([128, D], f32)
                for nb in range(2):
                    ps = ps_pool.tile([128, 512], f32)
                    for k2 in range(4):
                        nc.tensor.matmul(
                            ps,
                            lhsT=ctrn[:, k2, :, mt * 128:(mt + 1) * 128],
                            rhs=wo_sb[:, k2, :, nb * 512:(nb + 1) * 512],
                            start=(k2 == 0), stop=(k2 == 3), perf_mode=DR)
                    nc.vector.scalar_tensor_tensor(
                        out=hid[:, nb * 512:(nb + 1) * 512], in0=ps,
                        scalar=1.0 / 1024,
                        in1=xp[:, nb * 512:(nb + 1) * 512],
                        op0=mybir.AluOpType.mult,
                        op1=mybir.AluOpType.add)
                st = stat_pool.tile([128, 2, 6], f32, tag="st")
                for g in range(2):
                    nc.vector.bn_stats(out=st[:, g, :],
                                       in_=hid[:, g * 512:(g + 1) * 512])
                mv = stat_pool.tile([128, 2], f32, tag="mv")
                nc.vector.bn_aggr(out=mv, in_=st)
                sd = stat_pool.tile([128, 1], f32, tag="sd")
                nc.scalar.activation(out=sd, in_=mv[:, 1:2],
                                     func=mybir.ActivationFunctionType.Sqrt,
                                     bias=eps_sb, scale=1.0)
                rs = stat_pool.tile([128, 1], f32, tag="rs")
                nc.vector.reciprocal(out=rs, in_=sd)
                nmr = stat_pool.tile([128, 1], f32, tag="nmr")
                nc.vector.tensor_scalar(out=nmr, in0=mv[:, 0:1],
                                        scalar1=rs, scalar2=-1.0,
                                        op0=mybir.AluOpType.mult,
                                        op1=mybir.AluOpType.mult)
                ot = out_pool.tile([128, D], bf16)
                nc.scalar.activation(
                    out=ot, in_=hid,
                    func=mybir.ActivationFunctionType.Identity,
                    bias=nmr, scale=rs)
                if not ln_trivial:
                    nc.vector.tensor_mul(out=ot, in0=ot, in1=gm_sb)
                    nc.vector.tensor_add(out=ot, in0=ot, in1=bt_sb)
                engs[mt % 2].dma_start(
                    out=out[mt * 128:(mt + 1) * 128, :], in_=ot)

    nc.compile()
    return nc


def _get(name, **kw):
    key = (name, tuple(sorted(kw.items())))
    if key not in _cache:
        _cache[key] = (_build_stage_a(**kw) if name == "a"
                       else _build_stage_b(**kw))
    return _cache[key]


def _run(nc, in_maps, label):
    kwargs = {}
    if PROFILE:
        kwargs = dict(trace=True)
    res = run_bass_kernel_spmd(nc, in_maps, list(range(NCORES)), **kwargs)
    if PROFILE:
        last_exec_ns[label] = res.exec_time_ns
    return res.results


def kernel(**inputs):
    x = np.asarray(inputs["input_tensor"], dtype=np.float32)
    mask = np.asarray(inputs["attention_mask"], dtype=np.float32)[:, 0, 0, :]
    Wq = np.asarray(inputs["Wq"], dtype=np.float32)
    bq = np.asarray(inputs["bq"], dtype=np.float32)
    Wk = np.asarray(inputs["Wk"], dtype=np.float32)
    bk = np.asarray(inputs["bk"], dtype=np.float32)
    Wv = np.asarray(inputs["Wv"], dtype=np.float32)
    bv = np.asarray(inputs["bv"], dtype=np.float32)
    Wo = np.asarray(inputs["Wo"], dtype=np.float32)
    bo = np.asarray(inputs["bo"], dtype=np.float32)
    gamma = np.asarray(inputs["ln_gamma"], dtype=np.float32)
    beta = np.asarray(inputs["ln_beta"], dtype=np.float32)

    qkv_trivial = bool(np.all(bq == 0) and np.all(bk == 0)
                       and np.all(bv == 0))

    xf = x.reshape(R, D)
    # [rb, p, k2, slab, row]: element = xf[rb*512+row, k2*256+slab*128+p]
    xts8 = np.ascontiguousarray(
        xf.reshape(8, 512, 4, 2, 128).transpose(0, 4, 2, 3, 1)).astype(E4M3)
    mask_h = np.ascontiguousarray(
        mask.reshape(B, 16, 128).transpose(2, 0, 1))
    mask_sch = (SCH_ADD + 8 * LOG2E * mask_h).astype(np.float32)
    id8 = np.eye(128, dtype=BF16)

    def wprep(Wc):  # [1024, ncol] -> [128, 4, 2, ncol] fp8, x8 scale
        ncol = Wc.shape[1]
        return np.ascontiguousarray(
            (8 * Wc).reshape(4, 2, 128, ncol).transpose(2, 0, 1, 3)
        ).astype(E4M3)

    in_maps_a = []
    for c in range(NCORES):
        cs = slice(c * 128, (c + 1) * 128)
        in_maps_a.append({
            "xts": xts8,
            "wqk": wprep(np.concatenate([Wq[:, cs], Wk[:, cs]], axis=1)),
            "wv": wprep(Wv[:, cs]),
            "bqk": np.ascontiguousarray(
                8 * np.stack([bq[cs], bk[cs]], axis=1)).astype(np.float32),
            "bv": np.ascontiguousarray(8 * bv[cs, None]).astype(np.float32),
            "mask_e": mask_h,
            "mask_s": mask_sch,
            "id8": id8,
        })
    res_a = _run(_get("a", qkv_trivial=qkv_trivial), in_maps_a, "stage_a")

    # Assemble unnormalized ctx (value 8*U, bf16) and denominator rows.
    ctr_full = np.empty((D, R), dtype=np.float32)
    recb_full = np.empty((D, R), dtype=np.float32)
    for c in range(NCORES):
        ct = res_a[c]["ctxT"]  # [2, 65, R] f32
        for h in range(HPC):
            rows = slice((2 * c + h) * 64, (2 * c + h + 1) * 64)
            ctr_full[rows] = ct[h, 0:64] * (1.0 / 128.0)   # = U/16
            recb_full[rows] = 1024.0 / ct[h, 64]           # per-(h,q) recip
    ctr8_full = ctr_full.astype(E4M3)
    recb_full = recb_full.astype(E4M3)

    wo_b = np.ascontiguousarray(
        (16 * Wo).reshape(4, 2, 128, D).transpose(2, 0, 1, 3)).astype(E4M3)
    xpb_f = (xf + bo[None, :]).astype(BF16)
    ln_trivial = bool(np.all(gamma == 1.0) and np.all(beta == 0.0))

    def bprep(a, rs):  # [1024, R] -> [128, 4, 2, 512] core slice
        return np.ascontiguousarray(
            a[:, rs].reshape(4, 2, 128, RPC).transpose(2, 0, 1, 3))

    in_maps_b = []
    for c in range(NCORES):
        rs = slice(c * RPC, (c + 1) * RPC)
        in_maps_b.append({
            "ctr": bprep(ctr8_full, rs),
            "recb": bprep(recb_full, rs),
            "wo": wo_b,
            "xpb": np.ascontiguousarray(xpb_f[rs]),
            "gamma": gamma,
            "beta": beta,
        })
    res_b = _run(_get("b", ln_trivial=ln_trivial), in_maps_b, "stage_b")

    out = np.concatenate(
        [np.asarray(res_b[c]["out"], dtype=np.float32)
         for c in range(NCORES)], axis=0)
    return out.reshape(B, S, D)
